# revision 49
# baseline (speedup 1.0000x reference)
"""Trainium2 Bass kernel for a 4-layer LSTM decoder step with Bahdanau attention.

Math (B=128 batch, S=128 enc positions, H=A=E_enc=1024, emb=64, V=32000, NL=4):
  x   = E[tokens]
  o1  = LSTM_f([x, context], hidden0, cell0)
  ad  = o1 @ Wad.T + bad ; scores[s,b] = (enc @ Wae.T + bae)[s,b,:] . ad[b,:]
  ctx = softmax_s(scores)-weighted sum of enc over s
  h   = LSTM_l0([o1, ctx]) -> LSTM_r1(h) -> LSTM_r2(h)
  out = [h, ctx] @ Wout.T + bout                               # [128, 32000]

Distribution over 8 NeuronCores — ZERO collectives:
  - The LSTM stack and the attention block are small; they are fully
    replicated on every core (bf16 weights streamed from DRAM, fp32
    accumulation).  This removes every cross-core sync point: the per-core
    programs are dispatched with multi-ms skew, and any collective makes
    every core's device-side span absorb that skew.  Replication costs
    ~133MB of per-core DRAM reads (~370us at 358GB/s) instead.
  - Output projection is vocab-sharded: each core computes 4000 of the
    32000 logits (padded to 4096); shards are concatenated on the host.
  - scores use the identity  scores[s,b] = enc[s,b,:].(ad@Wae)[b,:]
    + ad[b,:].bae, so the [S,B,128] "ae" tensor is never materialized.
    Softmax uses the exp-without-max trick (scores are in [-10,10] at
    this model scale); one fused DVE op per position produces the score
    (accum_out) and one more accumulates exp-weighted context.
  - The attention phase is DVE-serial (~128x2 fused ops), so every
    input-independent gate contraction (l0's o1/W_hh parts, r1/r2's W_hh
    parts) is computed DURING it: those PSUM groups close early, park
    their partial sums in SBUF (bf16), and reopen with an identity-matmul
    reload once ctx / h2 / h3 arrive.  This keeps the weight DMA stream
    (the roofline resource) running through the attention window.

All activations live in transposed [feature, batch] layout for matmul
stationaries; gates are computed activation-stationary into [batch, hid]
PSUM tiles (one accumulation group per bank), bias folded in by an
opening ones-matmul; h is PE-transposed back to [128, B] chunks.
"""
import os
import sys

sys.path.insert(0, "/opt/trn_rl_repo")

WSTREAM_BUFS = int(os.environ.get("K_WSTREAM_BUFS", "7"))
ENC_BUFS = int(os.environ.get("K_ENC_BUFS", "8"))
WOUT_BUFS = int(os.environ.get("K_WOUT_BUFS", "2"))
SKIP_ATTN = int(os.environ.get("K_SKIP_ATTN", "0"))
POOL_FRAC = int(os.environ.get("K_POOL_FRAC", "3"))

import numpy as np
import ml_dtypes

from concourse import bacc, masks, mybir, tile
from concourse.bass_utils import run_bass_kernel_spmd

F32 = mybir.dt.float32
BF16 = mybir.dt.bfloat16
ALU = mybir.AluOpType
ACT = mybir.ActivationFunctionType
NPBF = ml_dtypes.bfloat16

B = 128          # batch
S = 128          # encoder length
H = 1024         # hidden dim
NL = 4           # LSTM layers
KATT = 128       # attention projection size
E = 1024         # encoder hidden dim
NCORES = 8
VSH = 32000 // NCORES    # 4000: vocab shard
VPAD = 4096              # padded vocab shard (8 x 512)
XC = 1152                # padded [emb(64) + context(1024)] input width (9 x 128)
KIN = [XC // 128, (2 * H) // 128, H // 128, H // 128]   # wih k-chunks per layer
NM = H // 128            # 8 hidden chunks per layer

_compiled = None


def _build():
    nc = bacc.Bacc("TRN2", target_bir_lowering=False, debug=False,
                   num_devices=NCORES)

    def din(name, shape, dt=BF16):
        return nc.dram_tensor(name, list(shape), dt, kind="ExternalInput").ap()

    xc_d = din("xc", [128, XC])               # packed [x, context] chunks
    hT_d = din("hT", [128, NL * H])           # packed prev hidden chunks
    cB_d = din("cB", [B, NL * H])             # prev cell, [batch, l*H+hid]
    wih = [din(f"wih{l}", [KIN[l], 2, 128, 2048]) for l in range(NL)]
    whh = [din(f"whh{l}", [NM, 2, 128, 2048]) for l in range(NL)]
    bias_d = din("bias", [NL * 2, 2048])      # per (l, half): g*512+q
    wad_d = din("wad", [128, H])              # packed Wad.T chunks
    bad_d = din("bad", [KATT, 1], F32)
    wae_d = din("wae", [KATT, E])
    bae_d = din("bae", [KATT, 1])
    enc_d = din("enc", [S, B, E])             # full encoder outputs
    wout = din("wout", [8, 128, 16 * 512])    # [vblock, k, kchunk*v]
    bout_d = din("bout", [1, VPAD])
    out = nc.dram_tensor("out", [B, VPAD], F32, kind="ExternalOutput").ap()

    with tile.TileContext(nc) as tc:
        with tc.tile_pool(name="const", bufs=1) as const, \
             tc.tile_pool(name="acts", bufs=1) as acts, \
             tc.tile_pool(name="wstream", bufs=1) as wstream, \
             tc.tile_pool(name="biasp", bufs=1) as biasp, \
             tc.tile_pool(name="stashp", bufs=1) as stashp, \
             tc.tile_pool(name="encp", bufs=1) as encp, \
             tc.tile_pool(name="scratch", bufs=1) as scratch, \
             tc.tile_pool(name="woutp", bufs=1) as woutp, \
             tc.tile_pool(name="gps", bufs=1, space="PSUM") as gps, \
             tc.tile_pool(name="atps", bufs=1, space="PSUM") as atps:

            # ---- constants / packed small inputs ----
            ident = const.tile([128, 128], BF16, tag="ident")
            masks.make_identity(nc, ident[:])
            ones = const.tile([1, 128], BF16, tag="ones")
            nc.vector.memset(ones[:], 1.0)
            bad_sb = const.tile([KATT, 1], F32, tag="bad")
            nc.sync.dma_start(bad_sb[:], bad_d[:])
            bae_sb = const.tile([KATT, 1], BF16, tag="bae")
            nc.sync.dma_start(bae_sb[:], bae_d[:])
            wae_sb = const.tile([KATT, E], BF16, tag="wae")
            nc.sync.dma_start(wae_sb[:], wae_d[:])
            wad_sb = const.tile([128, H], BF16, tag="wad")
            nc.sync.dma_start(wad_sb[:], wad_d[:])
            bout_sb = const.tile([1, VPAD], BF16, tag="bout")
            nc.sync.dma_start(bout_sb[:], bout_d[:])
            xc_sb = const.tile([128, XC], BF16, tag="xc")
            nc.sync.dma_start(xc_sb[:], xc_d[:])
            hT_sb = const.tile([128, NL * H], BF16, tag="hT")
            nc.sync.dma_start(hT_sb[:], hT_d[:])
            cB_sb = const.tile([B, NL * H], BF16, tag="cB")
            nc.sync.dma_start(cB_sb[:], cB_d[:])

            def h_prev(l, m):
                return hT_sb[:, (l * NM + m) * 128:(l * NM + m + 1) * 128]

            def wih_srcs(l, xts, lo=0, hi=None):
                hi = len(xts) if hi is None else hi
                return [(wih[l], k, xts[k]) for k in range(lo, hi)]

            def whh_srcs(l):
                return [(whh[l], k, h_prev(l, k)) for k in range(NM)]

            # ---- gate-group helpers ----
            def open_groups(l, half, stash=None):
                ps = [gps.tile([B, 512], F32, tag=f"gps{g}", bufs=1,
                               name=f"gps{g}") for g in range(4)]
                if stash is None:
                    bt = biasp.tile([1, 2048], BF16, tag="biasp", bufs=2,
                                    name="biasp")
                    nc.sync.dma_start(bt[:], bias_d[l * 2 + half:
                                                    l * 2 + half + 1, :])
                    for g in range(4):
                        nc.tensor.matmul(ps[g][:], ones[:],
                                         bt[:, g * 512:(g + 1) * 512],
                                         start=True, stop=False)
                else:
                    for g in range(4):
                        nc.tensor.matmul(ps[g][:], ident[:],
                                         stash[:, g * 512:(g + 1) * 512],
                                         start=True, stop=False)
                return ps

            def contract(ps, half, srcs, close):
                n = len(srcs)
                for k, (wsrc, wk, lhsT) in enumerate(srcs):
                    wt = wstream.tile([128, 2048], BF16, tag="wstream",
                                      bufs=WSTREAM_BUFS, name="wstream")
                    nc.sync.dma_start(wt[:], wsrc[wk, half])
                    for g in range(4):
                        nc.tensor.matmul(ps[g][:], lhsT,
                                         wt[:, g * 512:(g + 1) * 512],
                                         start=False,
                                         stop=(close and k == n - 1))

            def gates_part1(l, half, srcs):
                """Input-independent partial gate sums -> bf16 SBUF stash."""
                ps = open_groups(l, half)
                contract(ps, half, srcs, close=True)
                st = stashp.tile([B, 2048], BF16, tag="stash", bufs=6,
                                 name="stash")
                for g in range(4):
                    nc.scalar.activation(st[:, g * 512:(g + 1) * 512],
                                         ps[g][:], ACT.Copy)
                return st

            def gates_part2(l, half, srcs, stash):
                """Finish gates for (l, half) and run the LSTM elementwise.
                Returns the 4 h-chunk tiles [128, B] bf16 of this half."""
                ps = open_groups(l, half, stash=stash)
                contract(ps, half, srcs, close=True)
                si = scratch.tile([B, 512], F32, tag="ew", bufs=5, name="ew")
                sf = scratch.tile([B, 512], F32, tag="ew", bufs=5, name="ew")
                tg = scratch.tile([B, 512], F32, tag="ew", bufs=5, name="ew")
                so = scratch.tile([B, 512], F32, tag="ew", bufs=5, name="ew")
                nc.scalar.activation(si[:], ps[0][:], ACT.Sigmoid)
                nc.scalar.activation(sf[:], ps[1][:], ACT.Sigmoid)
                nc.scalar.activation(tg[:], ps[2][:], ACT.Tanh)
                nc.scalar.activation(so[:], ps[3][:], ACT.Sigmoid)
                c_half = cB_sb[:, l * H + half * 512:l * H + half * 512 + 512]
                t1 = scratch.tile([B, 512], F32, tag="ew", bufs=5, name="ew")
                t2 = scratch.tile([B, 512], F32, tag="ew", bufs=5, name="ew")
                nc.vector.tensor_tensor(t1[:], sf[:], c_half, ALU.mult)
                nc.vector.tensor_tensor(t2[:], si[:], tg[:], ALU.mult)
                c2 = scratch.tile([B, 512], F32, tag="ew", bufs=5, name="ew")
                nc.vector.tensor_tensor(c2[:], t1[:], t2[:], ALU.add)
                tc2 = scratch.tile([B, 512], F32, tag="ew", bufs=5, name="ew")
                nc.scalar.activation(tc2[:], c2[:], ACT.Tanh)
                hh = scratch.tile([B, 512], BF16, tag="hhalf", bufs=2,
                                  name="hhalf")
                nc.vector.tensor_tensor(hh[:], so[:], tc2[:], ALU.mult)
                hs = []
                for mm in range(4):
                    m = half * 4 + mm
                    tp = atps.tile([128, B], BF16, tag="tr", bufs=1, name="tr")
                    nc.tensor.transpose(tp[:], hh[:, mm * 128:(mm + 1) * 128],
                                        ident[:])
                    hm = acts.tile([128, B], BF16, tag=f"h{l}_{m}", bufs=1,
                                   name=f"h{l}_{m}")
                    nc.vector.tensor_copy(hm[:], tp[:])
                    hs.append(hm)
                return hs

            # ---- layer f (everything available immediately) ----
            xc_chunks = [xc_sb[:, k * 128:(k + 1) * 128] for k in range(KIN[0])]
            lf_srcs = wih_srcs(0, xc_chunks) + whh_srcs(0)
            h1 = (gates_part2(0, 0, lf_srcs, None)
                  + gates_part2(0, 1, lf_srcs, None))

            # ---- attention prologue (gates the s-loop; emitted first so the
            # PE runs it before the parked part1 contractions) ----
            ad_ps = atps.tile([128, B], F32, tag="adps", bufs=1, name="adps")
            for k in range(NM):
                nc.tensor.matmul(ad_ps[:], wad_sb[:, k * 128:(k + 1) * 128],
                                 h1[k][:], start=(k == 0), stop=(k == NM - 1))
            adT_sb = acts.tile([KATT, B], BF16, tag="adT")
            nc.scalar.activation(adT_sb[:], ad_ps[:], ACT.Identity, bias=bad_sb[:])
            w_sb = acts.tile([B, E], BF16, tag="w_att")
            for hf in range(2):
                wps = atps.tile([B, 512], F32, tag="psB", bufs=2, name="wps")
                nc.tensor.matmul(wps[:], adT_sb[:],
                                 wae_sb[:, hf * 512:(hf + 1) * 512],
                                 start=True, stop=True)
                nc.vector.tensor_copy(w_sb[:, hf * 512:(hf + 1) * 512], wps[:])
            c_ps = atps.tile([B, 1], F32, tag="adps", bufs=1, name="cdps")
            nc.tensor.matmul(c_ps[:], adT_sb[:], bae_sb[:], start=True, stop=True)
            cdot = acts.tile([B, 1], F32, tag="cdot")
            nc.vector.tensor_copy(cdot[:], c_ps[:])

            # ---- input-independent gate contractions, parked to overlap the
            # attention s-loop (32MB of weight stream + PE work) ----
            st_l0 = [gates_part1(1, hf, wih_srcs(1, h1, 0, NM) + whh_srcs(1))
                     for hf in range(2)]
            st_r1 = [gates_part1(2, hf, whh_srcs(2)) for hf in range(2)]
            st_r2 = [gates_part1(3, hf, whh_srcs(3)) for hf in range(2)]

            # ---- attention s-loop (DVE-serial; one fused op for the score,
            # one for the exp-weighted context accumulation) ----
            scoresb = acts.tile([B, S], F32, tag="scoresb")
            alphas = acts.tile([B, S], F32, tag="alphas")
            ctxs = acts.tile([B, E], F32, tag="ctxs")
            ctxp = acts.tile([B, E], F32, tag="ctxp")
            nc.vector.memset(ctxs[:], 0.0)
            nc.gpsimd.memset(ctxp[:], 0.0)
            nc.vector.memset(alphas[:], 1.0)
            SBLK = 2
            for sb in ([] if SKIP_ATTN else range(S // SBLK)):
                eb = encp.tile([B, SBLK * E], BF16, tag="enc", bufs=ENC_BUFS,
                               name="enc")
                nc.scalar.dma_start(eb[:].rearrange("b (s e) -> b s e", s=SBLK),
                                    enc_d[sb * SBLK:(sb + 1) * SBLK]
                                    .transpose([1, 0, 2]))
                for j in range(SBLK):
                    s = sb * SBLK + j
                    prod = scratch.tile([B, E], BF16, tag="prod", bufs=2,
                                        name="prod")
                    nc.vector.scalar_tensor_tensor(
                        prod[:], eb[:, j * E:(j + 1) * E], 1.0, w_sb[:],
                        ALU.mult, ALU.mult, accum_out=scoresb[:, s:s + 1])
                blk = slice(sb * SBLK, (sb + 1) * SBLK)
                nc.scalar.activation(alphas[:, blk], scoresb[:, blk],
                                     ACT.Exp, bias=cdot[:])
                for j in range(SBLK):
                    s = sb * SBLK + j
                    # Three-engine split: DVE keeps a few positions via the
                    # fused op; for the rest, ACT scales (Copy w/ per-partition
                    # alpha) and the idle Pool engine accumulates.
                    if s % 8 < POOL_FRAC:
                        nc.vector.scalar_tensor_tensor(
                            ctxs[:], eb[:, j * E:(j + 1) * E],
                            alphas[:, s:s + 1], ctxs[:], ALU.mult, ALU.add)
                    else:
                        wenc = scratch.tile([B, E], BF16, tag="wenc", bufs=2,
                                            name="wenc")
                        nc.scalar.activation(wenc[:], eb[:, j * E:(j + 1) * E],
                                             ACT.Copy, scale=alphas[:, s:s + 1])
                        nc.gpsimd.tensor_tensor(ctxp[:], ctxp[:], wenc[:],
                                                ALU.add)
            sumexp = acts.tile([B, 1], F32, tag="sumexp")
            nc.vector.tensor_reduce(sumexp[:], alphas[:], mybir.AxisListType.X,
                                    ALU.add)
            ctxa = acts.tile([B, E], F32, tag="ctxa")
            nc.vector.tensor_tensor(ctxa[:], ctxs[:], ctxp[:], ALU.add)
            recip = acts.tile([B, 1], F32, tag="recip")
            nc.vector.reciprocal(recip[:], sumexp[:])
            ctx_n = acts.tile([B, E], BF16, tag="ctx_n")
            nc.scalar.activation(ctx_n[:], ctxa[:], ACT.Copy, scale=recip[:])
            ctxT = []
            for k in range(NM):
                tp = atps.tile([128, B], BF16, tag="tr", bufs=1, name="tr")
                nc.tensor.transpose(tp[:], ctx_n[:, k * 128:(k + 1) * 128], ident[:])
                t = acts.tile([128, B], BF16, tag=f"ctxT{k}", bufs=1, name="ctxT")
                nc.vector.tensor_copy(t[:], tp[:])
                ctxT.append(t)

            # ---- finish layers l0, r1, r2 ----
            l0_late = [(wih[1], NM + k, ctxT[k][:]) for k in range(NM)]
            h2 = (gates_part2(1, 0, l0_late, st_l0[0])
                  + gates_part2(1, 1, l0_late, st_l0[1]))
            r1_late = wih_srcs(2, h2)
            h3 = (gates_part2(2, 0, r1_late, st_r1[0])
                  + gates_part2(2, 1, r1_late, st_r1[1]))
            r2_late = wih_srcs(3, h3)
            h4 = (gates_part2(3, 0, r2_late, st_r2[0])
                  + gates_part2(3, 1, r2_late, st_r2[1]))

            # ---- output projection: out[b, v] = [h4, ctx] @ Wout.T + bout ----
            xt16 = [t[:] for t in h4] + [t[:] for t in ctxT]
            for vb in range(8):
                wt = woutp.tile([128, 16 * 512], BF16, tag="wout",
                                bufs=WOUT_BUFS, name="wout")
                nc.scalar.dma_start(wt[:], wout[vb])
                ps = atps.tile([B, 512], F32, tag="psB", bufs=2, name="ops")
                nc.tensor.matmul(ps[:], ones[:],
                                 bout_sb[:, vb * 512:(vb + 1) * 512],
                                 start=True, stop=False)
                for k in range(16):
                    nc.tensor.matmul(ps[:], xt16[k],
                                     wt[:, k * 512:(k + 1) * 512],
                                     start=False, stop=(k == 15))
                ot = scratch.tile([B, 512], F32, tag="ot", bufs=2, name="ot")
                nc.vector.tensor_copy(ot[:], ps[:])
                nc.scalar.dma_start(out[:, vb * 512:(vb + 1) * 512], ot[:])

    nc.compile()
    return nc


def _pack_chunks(a, nchunks):
    """[nchunks*128, C] -> [128, nchunks*C]: feature chunk c at cols c*C."""
    r, c = a.shape
    assert r == nchunks * 128
    return np.ascontiguousarray(
        a.reshape(nchunks, 128, c).transpose(1, 0, 2).reshape(128, nchunks * c))


def _pack_gates(WT, K):
    """WT [K*128, 4096] (cols = gate g*1024 + m*128 + q)
    -> [K, 2, 128, 2048] with half h cols = g*512 + (m-4h)*128 + q."""
    return np.ascontiguousarray(
        WT.reshape(K, 128, 4, 2, 4, 128).transpose(0, 3, 1, 2, 4, 5)
        .reshape(K, 2, 128, 2048))


def _prep_in_maps(inputs):
    f32 = lambda a: np.ascontiguousarray(np.asarray(a), dtype=np.float32)
    bf = lambda a: np.ascontiguousarray(np.asarray(a, dtype=np.float32)
                                        .astype(NPBF))
    tokens = np.asarray(inputs["tokens"]).astype(np.int64)
    Emb = f32(inputs["E"])
    context = f32(inputs["context"])
    hidden = f32(inputs["hidden"])
    cell = f32(inputs["cell"])
    enc_out = f32(inputs["enc_outputs"])

    x = Emb[tokens]                                        # [B, 64]
    xc = np.concatenate([x, context], axis=1)              # [B, 1088]
    xc = np.pad(xc, ((0, 0), (0, XC - xc.shape[1])))       # [B, 1152]
    xc_p = bf(_pack_chunks(xc.T, XC // 128))               # [128, 1152]

    hT_p = bf(hidden.reshape(NL, B, NM, 128).transpose(3, 0, 2, 1)
              .reshape(128, NL * H))
    cB_p = bf(cell.transpose(1, 0, 2).reshape(B, NL * H))

    wih_full = [f32(inputs["W_ih_f"]), f32(inputs["W_ih_l0"]),
                f32(inputs["W_ih_rest"])[0], f32(inputs["W_ih_rest"])[1]]
    whh_full = [f32(inputs["W_hh_f"]), f32(inputs["W_hh_l0"]),
                f32(inputs["W_hh_rest"])[0], f32(inputs["W_hh_rest"])[1]]
    b_full = [f32(inputs["b_ih_f"]) + f32(inputs["b_hh_f"]),
              f32(inputs["b_ih_l0"]) + f32(inputs["b_hh_l0"]),
              f32(inputs["b_ih_rest"])[0] + f32(inputs["b_hh_rest"])[0],
              f32(inputs["b_ih_rest"])[1] + f32(inputs["b_hh_rest"])[1]]

    shared = {"xc": xc_p, "hT": hT_p, "cB": cB_p}
    for l in range(NL):
        WT = wih_full[l].T                                 # [in, 4096]
        if l == 0:
            WT = np.pad(WT, ((0, XC - WT.shape[0]), (0, 0)))
        shared[f"wih{l}"] = bf(_pack_gates(WT, KIN[l]))
        shared[f"whh{l}"] = bf(_pack_gates(whh_full[l].T, NM))
    # bias rows per (l, half): col = g*512 + q, value b[g*1024 + half*512 + q]
    shared["bias"] = bf(np.stack(b_full).reshape(NL, 4, 2, 512)
                        .transpose(0, 2, 1, 3).reshape(NL * 2, 2048))
    shared["wad"] = bf(_pack_chunks(f32(inputs["Wad"]).T, NM))   # [128, 1024]
    shared["bad"] = f32(inputs["bad"]).reshape(KATT, 1)
    shared["wae"] = bf(inputs["Wae"])                      # [128, E]
    shared["bae"] = bf(np.asarray(inputs["bae"]).reshape(KATT, 1))
    shared["enc"] = bf(enc_out)                            # [S, B, E]

    Wout = f32(inputs["Wout"])
    bout_full = f32(inputs["bout"])
    in_maps = []
    for c in range(NCORES):
        Wsh = Wout[c * VSH:(c + 1) * VSH]                  # [4000, 2048]
        Wsh = np.pad(Wsh, ((0, VPAD - VSH), (0, 0)))       # [4096, 2048]
        WT = Wsh.T                                         # [2048, 4096]
        wout_p = bf(WT.reshape(16, 128, 8, 512).transpose(2, 1, 0, 3)
                    .reshape(8, 128, 16 * 512))
        m = dict(shared)
        m["wout"] = wout_p
        m["bout"] = bf(np.pad(bout_full[c * VSH:(c + 1) * VSH],
                              (0, VPAD - VSH)).reshape(1, VPAD))
        in_maps.append(m)
    return in_maps


def get_compiled():
    global _compiled
    if _compiled is None:
        _compiled = _build()
    return _compiled


def kernel(**inputs):
    nc = get_compiled()
    in_maps = _prep_in_maps(inputs)
    res = run_bass_kernel_spmd(nc, in_maps, core_ids=list(range(NCORES)))
    out = np.concatenate([res.results[c]["out"][:, :VSH] for c in range(NCORES)],
                         axis=1)
    return out


# revision 53
# speedup vs baseline: 14.5586x; 14.5586x over previous
"""Trainium2 Bass kernel for a 4-layer LSTM decoder step with Bahdanau attention.

Math (B=128 batch, S=128 enc positions, H=A=E_enc=1024, emb=64, V=32000, NL=4):
  x   = E[tokens]
  o1  = LSTM_f([x, context], hidden0, cell0)
  ad  = o1 @ Wad.T + bad ; scores[s,b] = (enc @ Wae.T + bae)[s,b,:] . ad[b,:]
  ctx = softmax_s(scores)-weighted sum of enc over s
  h   = LSTM_l0([o1, ctx]) -> LSTM_r1(h) -> LSTM_r2(h)
  out = [h, ctx] @ Wout.T + bout                               # [128, 32000]

Distribution over 8 NeuronCores — ZERO collectives:
  - The LSTM stack and the attention block are small; they are fully
    replicated on every core (bf16 weights streamed from DRAM, fp32
    accumulation).  This removes every cross-core sync point: the per-core
    programs are dispatched with multi-ms skew, and any collective makes
    every core's device-side span absorb that skew.  Replication costs
    ~133MB of per-core DRAM reads (~370us at 358GB/s) instead.
  - Output projection is vocab-sharded: each core computes 4000 of the
    32000 logits (padded to 4096); shards are concatenated on the host.
  - scores use the identity  scores[s,b] = enc[s,b,:].(ad@Wae)[b,:]
    + ad[b,:].bae, so the [S,B,128] "ae" tensor is never materialized.
    Softmax uses the exp-without-max trick (scores are in [-10,10] at
    this model scale); one fused DVE op per position produces the score
    (accum_out) and one more accumulates exp-weighted context.
  - The attention phase is DVE-serial (~128x2 fused ops), so every
    input-independent gate contraction (l0's o1/W_hh parts, r1/r2's W_hh
    parts) is computed DURING it: those PSUM groups close early, park
    their partial sums in SBUF (bf16), and reopen with an identity-matmul
    reload once ctx / h2 / h3 arrive.  This keeps the weight DMA stream
    (the roofline resource) running through the attention window.

All activations live in transposed [feature, batch] layout for matmul
stationaries; gates are computed activation-stationary into [batch, hid]
PSUM tiles (one accumulation group per bank), bias folded in by an
opening ones-matmul; h is PE-transposed back to [128, B] chunks.
"""
import os
import sys

sys.path.insert(0, "/opt/trn_rl_repo")

WSTREAM_BUFS = int(os.environ.get("K_WSTREAM_BUFS", "7"))
ENC_BUFS = int(os.environ.get("K_ENC_BUFS", "8"))
WOUT_BUFS = int(os.environ.get("K_WOUT_BUFS", "2"))
SKIP_ATTN = int(os.environ.get("K_SKIP_ATTN", "0"))
POOL_FRAC = int(os.environ.get("K_POOL_FRAC", "3"))

import numpy as np
import ml_dtypes

from concourse import bacc, masks, mybir, tile
from concourse.bass_utils import run_bass_kernel_spmd

F32 = mybir.dt.float32
BF16 = mybir.dt.bfloat16
ALU = mybir.AluOpType
ACT = mybir.ActivationFunctionType
NPBF = ml_dtypes.bfloat16

B = 128          # batch
S = 128          # encoder length
H = 1024         # hidden dim
NL = 4           # LSTM layers
KATT = 128       # attention projection size
E = 1024         # encoder hidden dim
NCORES = 8
VSH = 32000 // NCORES    # 4000: vocab shard
VPAD = 4096              # padded vocab shard (8 x 512)
XC = 1152                # padded [emb(64) + context(1024)] input width (9 x 128)
KIN = [XC // 128, (2 * H) // 128, H // 128, H // 128]   # wih k-chunks per layer
NM = H // 128            # 8 hidden chunks per layer

_compiled = None


def _build():
    nc = bacc.Bacc("TRN2", target_bir_lowering=False, debug=False,
                   num_devices=NCORES)

    def din(name, shape, dt=BF16):
        return nc.dram_tensor(name, list(shape), dt, kind="ExternalInput").ap()

    xc_d = din("xc", [128, XC])               # packed [x, context] chunks
    hT_d = din("hT", [128, NL * H])           # packed prev hidden chunks
    cB_d = din("cB", [B, NL * H])             # prev cell, [batch, l*H+hid]
    wih = [din(f"wih{l}", [KIN[l], 2, 128, 2048]) for l in range(NL)]
    whh = [din(f"whh{l}", [NM, 2, 128, 2048]) for l in range(NL)]
    bias_d = din("bias", [NL * 2, 2048])      # per (l, half): g*512+q
    wad_d = din("wad", [128, H])              # packed Wad.T chunks
    bad_d = din("bad", [KATT, 1], F32)
    wae_d = din("wae", [KATT, E])
    bae_d = din("bae", [KATT, 1])
    enc_d = din("enc", [S, B, E])             # full encoder outputs
    wout = din("wout", [8, 128, 16 * 512])    # [vblock, k, kchunk*v]
    bout_d = din("bout", [1, VPAD])
    out = nc.dram_tensor("out", [B, VPAD], F32, kind="ExternalOutput").ap()

    with tile.TileContext(nc) as tc:
        with tc.tile_pool(name="const", bufs=1) as const, \
             tc.tile_pool(name="acts", bufs=1) as acts, \
             tc.tile_pool(name="wstream", bufs=1) as wstream, \
             tc.tile_pool(name="biasp", bufs=1) as biasp, \
             tc.tile_pool(name="stashp", bufs=1) as stashp, \
             tc.tile_pool(name="encp", bufs=1) as encp, \
             tc.tile_pool(name="scratch", bufs=1) as scratch, \
             tc.tile_pool(name="woutp", bufs=1) as woutp, \
             tc.tile_pool(name="gps", bufs=1, space="PSUM") as gps, \
             tc.tile_pool(name="atps", bufs=1, space="PSUM") as atps:

            # ---- constants / packed small inputs ----
            ident = const.tile([128, 128], BF16, tag="ident")
            masks.make_identity(nc, ident[:])
            ones = const.tile([1, 128], BF16, tag="ones")
            nc.vector.memset(ones[:], 1.0)
            bad_sb = const.tile([KATT, 1], F32, tag="bad")
            nc.sync.dma_start(bad_sb[:], bad_d[:])
            bae_sb = const.tile([KATT, 1], BF16, tag="bae")
            nc.sync.dma_start(bae_sb[:], bae_d[:])
            wae_sb = const.tile([KATT, E], BF16, tag="wae")
            nc.sync.dma_start(wae_sb[:], wae_d[:])
            wad_sb = const.tile([128, H], BF16, tag="wad")
            nc.sync.dma_start(wad_sb[:], wad_d[:])
            bout_sb = const.tile([1, VPAD], BF16, tag="bout")
            nc.sync.dma_start(bout_sb[:], bout_d[:])
            xc_sb = const.tile([128, XC], BF16, tag="xc")
            nc.sync.dma_start(xc_sb[:], xc_d[:])
            hT_sb = const.tile([128, NL * H], BF16, tag="hT")
            nc.scalar.dma_start(hT_sb[:], hT_d[:])
            cB_sb = const.tile([B, NL * H], BF16, tag="cB")
            nc.scalar.dma_start(cB_sb[:], cB_d[:])

            def h_prev(l, m):
                return hT_sb[:, (l * NM + m) * 128:(l * NM + m + 1) * 128]

            def wih_srcs(l, xts, lo=0, hi=None):
                hi = len(xts) if hi is None else hi
                return [(wih[l], k, xts[k]) for k in range(lo, hi)]

            def whh_srcs(l):
                return [(whh[l], k, h_prev(l, k)) for k in range(NM)]

            # ---- gate-group helpers ----
            def open_groups(l, half, stash=None):
                ps = [gps.tile([B, 512], F32, tag=f"gps{g}", bufs=1,
                               name=f"gps{g}") for g in range(4)]
                if stash is None:
                    bt = biasp.tile([1, 2048], BF16, tag="biasp", bufs=2,
                                    name="biasp")
                    nc.sync.dma_start(bt[:], bias_d[l * 2 + half:
                                                    l * 2 + half + 1, :])
                    for g in range(4):
                        nc.tensor.matmul(ps[g][:], ones[:],
                                         bt[:, g * 512:(g + 1) * 512],
                                         start=True, stop=False)
                else:
                    for g in range(4):
                        nc.tensor.matmul(ps[g][:], ident[:],
                                         stash[:, g * 512:(g + 1) * 512],
                                         start=True, stop=False)
                return ps

            def contract(ps, half, srcs, close):
                n = len(srcs)
                for k, (wsrc, wk, lhsT) in enumerate(srcs):
                    wt = wstream.tile([128, 2048], BF16, tag="wstream",
                                      bufs=WSTREAM_BUFS, name="wstream")
                    nc.sync.dma_start(wt[:], wsrc[wk, half])
                    for g in range(4):
                        nc.tensor.matmul(ps[g][:], lhsT,
                                         wt[:, g * 512:(g + 1) * 512],
                                         start=False,
                                         stop=(close and k == n - 1))

            def gates_part1(l, half, srcs):
                """Input-independent partial gate sums -> bf16 SBUF stash."""
                ps = open_groups(l, half)
                contract(ps, half, srcs, close=True)
                st = stashp.tile([B, 2048], BF16, tag="stash", bufs=6,
                                 name="stash")
                for g in range(4):
                    nc.scalar.activation(st[:, g * 512:(g + 1) * 512],
                                         ps[g][:], ACT.Copy)
                return st

            def gates_part2(l, half, srcs, stash):
                """Finish gates for (l, half) and run the LSTM elementwise.
                Returns the 4 h-chunk tiles [128, B] bf16 of this half."""
                ps = open_groups(l, half, stash=stash)
                contract(ps, half, srcs, close=True)
                si = scratch.tile([B, 512], F32, tag="ew", bufs=5, name="ew")
                sf = scratch.tile([B, 512], F32, tag="ew", bufs=5, name="ew")
                tg = scratch.tile([B, 512], F32, tag="ew", bufs=5, name="ew")
                so = scratch.tile([B, 512], F32, tag="ew", bufs=5, name="ew")
                nc.scalar.activation(si[:], ps[0][:], ACT.Sigmoid)
                nc.scalar.activation(sf[:], ps[1][:], ACT.Sigmoid)
                nc.scalar.activation(tg[:], ps[2][:], ACT.Tanh)
                nc.scalar.activation(so[:], ps[3][:], ACT.Sigmoid)
                c_half = cB_sb[:, l * H + half * 512:l * H + half * 512 + 512]
                t1 = scratch.tile([B, 512], F32, tag="ew", bufs=5, name="ew")
                t2 = scratch.tile([B, 512], F32, tag="ew", bufs=5, name="ew")
                nc.vector.tensor_tensor(t1[:], sf[:], c_half, ALU.mult)
                nc.vector.tensor_tensor(t2[:], si[:], tg[:], ALU.mult)
                c2 = scratch.tile([B, 512], F32, tag="ew", bufs=5, name="ew")
                nc.vector.tensor_tensor(c2[:], t1[:], t2[:], ALU.add)
                tc2 = scratch.tile([B, 512], F32, tag="ew", bufs=5, name="ew")
                nc.scalar.activation(tc2[:], c2[:], ACT.Tanh)
                hh = scratch.tile([B, 512], BF16, tag="hhalf", bufs=2,
                                  name="hhalf")
                nc.vector.tensor_tensor(hh[:], so[:], tc2[:], ALU.mult)
                hs = []
                for mm in range(4):
                    m = half * 4 + mm
                    tp = atps.tile([128, B], BF16, tag="tr", bufs=1, name="tr")
                    nc.tensor.transpose(tp[:], hh[:, mm * 128:(mm + 1) * 128],
                                        ident[:])
                    hm = acts.tile([128, B], BF16, tag=f"h{l}_{m}", bufs=1,
                                   name=f"h{l}_{m}")
                    nc.vector.tensor_copy(hm[:], tp[:])
                    hs.append(hm)
                return hs

            # ---- layer f (everything available immediately) ----
            xc_chunks = [xc_sb[:, k * 128:(k + 1) * 128] for k in range(KIN[0])]
            lf_srcs = wih_srcs(0, xc_chunks) + whh_srcs(0)
            h1 = (gates_part2(0, 0, lf_srcs, None)
                  + gates_part2(0, 1, lf_srcs, None))

            # ---- attention prologue (gates the s-loop; emitted first so the
            # PE runs it before the parked part1 contractions) ----
            ad_ps = atps.tile([128, B], F32, tag="adps", bufs=1, name="adps")
            for k in range(NM):
                nc.tensor.matmul(ad_ps[:], wad_sb[:, k * 128:(k + 1) * 128],
                                 h1[k][:], start=(k == 0), stop=(k == NM - 1))
            adT_sb = acts.tile([KATT, B], BF16, tag="adT")
            nc.scalar.activation(adT_sb[:], ad_ps[:], ACT.Identity, bias=bad_sb[:])
            w_sb = acts.tile([B, E], BF16, tag="w_att")
            for hf in range(2):
                wps = atps.tile([B, 512], F32, tag="psB", bufs=2, name="wps")
                nc.tensor.matmul(wps[:], adT_sb[:],
                                 wae_sb[:, hf * 512:(hf + 1) * 512],
                                 start=True, stop=True)
                nc.vector.tensor_copy(w_sb[:, hf * 512:(hf + 1) * 512], wps[:])
            c_ps = atps.tile([B, 1], F32, tag="adps", bufs=1, name="cdps")
            nc.tensor.matmul(c_ps[:], adT_sb[:], bae_sb[:], start=True, stop=True)
            cdot = acts.tile([B, 1], F32, tag="cdot")
            nc.vector.tensor_copy(cdot[:], c_ps[:])

            # ---- input-independent gate contractions, parked to overlap the
            # attention s-loop (32MB of weight stream + PE work) ----
            st_l0 = [gates_part1(1, hf, wih_srcs(1, h1, 0, NM) + whh_srcs(1))
                     for hf in range(2)]
            st_r1 = [gates_part1(2, hf, whh_srcs(2)) for hf in range(2)]
            st_r2 = [gates_part1(3, hf, whh_srcs(3)) for hf in range(2)]

            # ---- attention s-loop (DVE-serial; one fused op for the score,
            # one for the exp-weighted context accumulation) ----
            scoresb = acts.tile([B, S], F32, tag="scoresb")
            alphas = acts.tile([B, S], F32, tag="alphas")
            ctxs = acts.tile([B, E], F32, tag="ctxs")
            ctxp = acts.tile([B, E], F32, tag="ctxp")
            nc.vector.memset(ctxs[:], 0.0)
            nc.gpsimd.memset(ctxp[:], 0.0)
            nc.vector.memset(alphas[:], 1.0)
            SBLK = 2
            for sb in ([] if SKIP_ATTN else range(S // SBLK)):
                eb = encp.tile([B, SBLK * E], BF16, tag="enc", bufs=ENC_BUFS,
                               name="enc")
                nc.scalar.dma_start(eb[:].rearrange("b (s e) -> b s e", s=SBLK),
                                    enc_d[sb * SBLK:(sb + 1) * SBLK]
                                    .transpose([1, 0, 2]))
                for j in range(SBLK):
                    s = sb * SBLK + j
                    prod = scratch.tile([B, E], BF16, tag="prod", bufs=2,
                                        name="prod")
                    nc.vector.scalar_tensor_tensor(
                        prod[:], eb[:, j * E:(j + 1) * E], 1.0, w_sb[:],
                        ALU.mult, ALU.mult, accum_out=scoresb[:, s:s + 1])
                blk = slice(sb * SBLK, (sb + 1) * SBLK)
                nc.scalar.activation(alphas[:, blk], scoresb[:, blk],
                                     ACT.Exp, bias=cdot[:])
                for j in range(SBLK):
                    s = sb * SBLK + j
                    # Three-engine split: DVE keeps a few positions via the
                    # fused op; for the rest, ACT scales (Copy w/ per-partition
                    # alpha) and the idle Pool engine accumulates.
                    if s % 8 < POOL_FRAC:
                        nc.vector.scalar_tensor_tensor(
                            ctxs[:], eb[:, j * E:(j + 1) * E],
                            alphas[:, s:s + 1], ctxs[:], ALU.mult, ALU.add)
                    else:
                        wenc = scratch.tile([B, E], BF16, tag="wenc", bufs=2,
                                            name="wenc")
                        nc.scalar.activation(wenc[:], eb[:, j * E:(j + 1) * E],
                                             ACT.Copy, scale=alphas[:, s:s + 1])
                        nc.gpsimd.tensor_tensor(ctxp[:], ctxp[:], wenc[:],
                                                ALU.add)
            sumexp = acts.tile([B, 1], F32, tag="sumexp")
            nc.vector.tensor_reduce(sumexp[:], alphas[:], mybir.AxisListType.X,
                                    ALU.add)
            ctxa = acts.tile([B, E], F32, tag="ctxa")
            nc.vector.tensor_tensor(ctxa[:], ctxs[:], ctxp[:], ALU.add)
            recip = acts.tile([B, 1], F32, tag="recip")
            nc.vector.reciprocal(recip[:], sumexp[:])
            ctx_n = acts.tile([B, E], BF16, tag="ctx_n")
            nc.scalar.activation(ctx_n[:], ctxa[:], ACT.Copy, scale=recip[:])
            ctxT = []
            for k in range(NM):
                tp = atps.tile([128, B], BF16, tag="tr", bufs=1, name="tr")
                nc.tensor.transpose(tp[:], ctx_n[:, k * 128:(k + 1) * 128], ident[:])
                t = acts.tile([128, B], BF16, tag=f"ctxT{k}", bufs=1, name="ctxT")
                nc.vector.tensor_copy(t[:], tp[:])
                ctxT.append(t)

            # ---- finish layers l0, r1, r2 ----
            l0_late = [(wih[1], NM + k, ctxT[k][:]) for k in range(NM)]
            h2 = (gates_part2(1, 0, l0_late, st_l0[0])
                  + gates_part2(1, 1, l0_late, st_l0[1]))
            r1_late = wih_srcs(2, h2)
            h3 = (gates_part2(2, 0, r1_late, st_r1[0])
                  + gates_part2(2, 1, r1_late, st_r1[1]))
            r2_late = wih_srcs(3, h3)
            h4 = (gates_part2(3, 0, r2_late, st_r2[0])
                  + gates_part2(3, 1, r2_late, st_r2[1]))

            # ---- output projection: out[b, v] = [h4, ctx] @ Wout.T + bout ----
            xt16 = [t[:] for t in h4] + [t[:] for t in ctxT]
            for vb in range(8):
                wt = woutp.tile([128, 16 * 512], BF16, tag="wout",
                                bufs=WOUT_BUFS, name="wout")
                nc.scalar.dma_start(wt[:], wout[vb])
                ps = atps.tile([B, 512], F32, tag="psB", bufs=2, name="ops")
                nc.tensor.matmul(ps[:], ones[:],
                                 bout_sb[:, vb * 512:(vb + 1) * 512],
                                 start=True, stop=False)
                for k in range(16):
                    nc.tensor.matmul(ps[:], xt16[k],
                                     wt[:, k * 512:(k + 1) * 512],
                                     start=False, stop=(k == 15))
                ot = scratch.tile([B, 512], F32, tag="ot", bufs=2, name="ot")
                nc.vector.tensor_copy(ot[:], ps[:])
                nc.scalar.dma_start(out[:, vb * 512:(vb + 1) * 512], ot[:])

    nc.compile()
    return nc


def _pack_chunks(a, nchunks):
    """[nchunks*128, C] -> [128, nchunks*C]: feature chunk c at cols c*C."""
    r, c = a.shape
    assert r == nchunks * 128
    return np.ascontiguousarray(
        a.reshape(nchunks, 128, c).transpose(1, 0, 2).reshape(128, nchunks * c))


def _pack_gates(WT, K):
    """WT [K*128, 4096] (cols = gate g*1024 + m*128 + q)
    -> [K, 2, 128, 2048] with half h cols = g*512 + (m-4h)*128 + q."""
    return np.ascontiguousarray(
        WT.reshape(K, 128, 4, 2, 4, 128).transpose(0, 3, 1, 2, 4, 5)
        .reshape(K, 2, 128, 2048))


def _prep_in_maps(inputs):
    f32 = lambda a: np.ascontiguousarray(np.asarray(a), dtype=np.float32)
    bf = lambda a: np.ascontiguousarray(np.asarray(a, dtype=np.float32)
                                        .astype(NPBF))
    tokens = np.asarray(inputs["tokens"]).astype(np.int64)
    Emb = f32(inputs["E"])
    context = f32(inputs["context"])
    hidden = f32(inputs["hidden"])
    cell = f32(inputs["cell"])
    enc_out = f32(inputs["enc_outputs"])

    x = Emb[tokens]                                        # [B, 64]
    xc = np.concatenate([x, context], axis=1)              # [B, 1088]
    xc = np.pad(xc, ((0, 0), (0, XC - xc.shape[1])))       # [B, 1152]
    xc_p = bf(_pack_chunks(xc.T, XC // 128))               # [128, 1152]

    hT_p = bf(hidden.reshape(NL, B, NM, 128).transpose(3, 0, 2, 1)
              .reshape(128, NL * H))
    cB_p = bf(cell.transpose(1, 0, 2).reshape(B, NL * H))

    wih_full = [f32(inputs["W_ih_f"]), f32(inputs["W_ih_l0"]),
                f32(inputs["W_ih_rest"])[0], f32(inputs["W_ih_rest"])[1]]
    whh_full = [f32(inputs["W_hh_f"]), f32(inputs["W_hh_l0"]),
                f32(inputs["W_hh_rest"])[0], f32(inputs["W_hh_rest"])[1]]
    b_full = [f32(inputs["b_ih_f"]) + f32(inputs["b_hh_f"]),
              f32(inputs["b_ih_l0"]) + f32(inputs["b_hh_l0"]),
              f32(inputs["b_ih_rest"])[0] + f32(inputs["b_hh_rest"])[0],
              f32(inputs["b_ih_rest"])[1] + f32(inputs["b_hh_rest"])[1]]

    shared = {"xc": xc_p, "hT": hT_p, "cB": cB_p}
    for l in range(NL):
        WT = wih_full[l].T                                 # [in, 4096]
        if l == 0:
            WT = np.pad(WT, ((0, XC - WT.shape[0]), (0, 0)))
        shared[f"wih{l}"] = bf(_pack_gates(WT, KIN[l]))
        shared[f"whh{l}"] = bf(_pack_gates(whh_full[l].T, NM))
    # bias rows per (l, half): col = g*512 + q, value b[g*1024 + half*512 + q]
    shared["bias"] = bf(np.stack(b_full).reshape(NL, 4, 2, 512)
                        .transpose(0, 2, 1, 3).reshape(NL * 2, 2048))
    shared["wad"] = bf(_pack_chunks(f32(inputs["Wad"]).T, NM))   # [128, 1024]
    shared["bad"] = f32(inputs["bad"]).reshape(KATT, 1)
    shared["wae"] = bf(inputs["Wae"])                      # [128, E]
    shared["bae"] = bf(np.asarray(inputs["bae"]).reshape(KATT, 1))
    shared["enc"] = bf(enc_out)                            # [S, B, E]

    Wout = f32(inputs["Wout"])
    bout_full = f32(inputs["bout"])
    in_maps = []
    for c in range(NCORES):
        Wsh = Wout[c * VSH:(c + 1) * VSH]                  # [4000, 2048]
        Wsh = np.pad(Wsh, ((0, VPAD - VSH), (0, 0)))       # [4096, 2048]
        WT = Wsh.T                                         # [2048, 4096]
        wout_p = bf(WT.reshape(16, 128, 8, 512).transpose(2, 1, 0, 3)
                    .reshape(8, 128, 16 * 512))
        m = dict(shared)
        m["wout"] = wout_p
        m["bout"] = bf(np.pad(bout_full[c * VSH:(c + 1) * VSH],
                              (0, VPAD - VSH)).reshape(1, VPAD))
        in_maps.append(m)
    return in_maps


def get_compiled():
    global _compiled
    if _compiled is None:
        _compiled = _build()
    return _compiled


def kernel(**inputs):
    nc = get_compiled()
    in_maps = _prep_in_maps(inputs)
    res = run_bass_kernel_spmd(nc, in_maps, core_ids=list(range(NCORES)))
    out = np.concatenate([res.results[c]["out"][:, :VSH] for c in range(NCORES)],
                         axis=1)
    return out
